# revision 19
# baseline (speedup 1.0000x reference)
"""Causal self-attention (B=2, T=2048, C=1024, H=16, D=64) on 8 trn2 NeuronCores.

Sharding: batch x head-group. Core c handles batch b = c//4 and head group
g = c%4 (4 heads = 256 channels). All matmul operands are bf16 (fp32 PSUM
accumulation); host pre-casts inputs to bf16, halving DMA and enabling FWL
so LDWEIGHTS pipelines behind matmuls.

Per core:
  - qkv projection for its 4 heads (Q^T/K^T in [d, t] layout, V in
    [t, d|ones|pad] layout padded to 128 weight columns for FWL)
  - causal flash attention, K-major scores S^T. Both heads of a pair are
    computed per 128-key chunk: two row-tiled (K=64) score matmuls into one
    2-bank PSUM tile, then ONE exp over [128, 2*512]:
      even chunks: ACT engine true exp (bf16 out)
      odd  chunks: DVE Schraudolph bit-trick exp (fp32->int16 mult+add,
                   int16 bits == bf16 exp approx, ~3% elementwise, washes
                   out through softmax normalization; end-to-end emulated
                   rel err 3.4e-3 vs 2e-2 budget)
    row sums via a ones-column appended to V; no max subtraction (logits
    are O(1) for this problem family).
  - AllGather of Y^T (bf16) across the 4 cores of the batch, one per head
    pair; first overlaps attention of the second pair.
  - output projection for a 256-column slice, accumulated pair-by-pair.

Host gather: concatenate the 4 column slices per batch. No host math.
"""

import os
import numpy as np
import ml_dtypes

import concourse.bass as bass
import concourse.bacc as bacc
import concourse.mybir as mybir
import concourse.tile as tile
from concourse import bass_utils
from concourse.bass import ds, ts
from concourse.alu_op_type import AluOpType
from concourse.bass_interp import get_hw_module

P = 128
B, T, C = 2, 2048, 1024
NH, D = 16, 64
NC = 8          # cores
NG = 4          # head groups (cores per batch)
HL = NH // NG   # heads per core = 4
DL = HL * D     # local channels = 256
NQ = 512        # query tile
F32 = mybir.dt.float32
BF16 = mybir.dt.bfloat16
I16 = mybir.dt.int16

SCALE = 1.0 / np.sqrt(D)
# Schraudolph fast-exp in bf16 bit space: bf16(exp(x)) bits ~= round(A*x + Bc)
A16 = 128.0 / np.log(2.0)
B16 = 127.0 * 128.0 - 5.5


def _build_body(ctx, tc, io, dbg=None):
    nc = tc.nc
    xt, wq, wk, wv, bq, bk, bv, wp, bp, tri, out, ytl, ytf = io
    mm = nc.tensor.matmul

    pers = ctx.enter_context(tc.tile_pool(name="pers", bufs=1))

    # ---------------- persistent SBUF + input loads ----------------
    tri_sb = pers.tile([P, P], BF16)
    nc.sync.dma_start(tri_sb[:], tri)

    qt_sb = pers.tile([P, 2, T], BF16)   # chunk j = head pair j
    kt_sb = pers.tile([P, 2, T], BF16)
    # V: [l_part, l_chunk, head, d | ones | zero-pad] -- 128 weight cols
    v_sb = pers.tile([P, T // P, HL, P], BF16)
    nc.vector.memset(v_sb[:, :, :, D:], 0.0)
    nc.vector.memset(v_sb[:, :, :, D : D + 1], 1.0)
    # [d, hi, pair, t]
    yth = pers.tile([D, 2, 2, T], BF16)

    xt_sb = pers.tile([P, C // P, T], BF16)
    wq_sb = pers.tile([P, C // P, DL], BF16)
    wk_sb = pers.tile([P, C // P, DL], BF16)
    wv_sb = pers.tile([P, C // P, DL], BF16)
    nc.sync.dma_start(wk_sb[:], wk.rearrange("(o p) n -> p o n", p=P))
    nc.sync.dma_start(wq_sb[:], wq.rearrange("(o p) n -> p o n", p=P))
    nc.sync.dma_start(wv_sb[:], wv.rearrange("(o p) n -> p o n", p=P))
    # T-tile-major so the first matmul groups complete after ~1/4 of the load
    for tt in range(T // NQ):
        nc.sync.dma_start(
            xt_sb[:, :, ts(tt, NQ)],
            xt[:, ds(NQ * tt, NQ)].rearrange("(cc p) t -> p cc t", p=P),
        )

    bqp = pers.tile([P, 2], F32)
    nc.sync.dma_start(bqp[:], bq.rearrange("(j p) -> p j", p=P))
    bkp = pers.tile([P, 2], F32)
    nc.sync.dma_start(bkp[:], bk.rearrange("(j p) -> p j", p=P))
    bv_row = pers.tile([1, DL], F32)
    nc.sync.dma_start(bv_row[:], bv[None, :])
    bv_bc = pers.tile([P, DL], F32)
    nc.gpsimd.partition_broadcast(bv_bc[:], bv_row[:])

    wp_sb = pers.tile([P, C // P, DL], BF16)
    nc.sync.dma_start(wp_sb[:], wp.rearrange("(o p) n -> p o n", p=P))
    bpp = pers.tile([P, 2], F32)
    nc.sync.dma_start(bpp[:], bp.rearrange("(j p) -> p j", p=P))

    # ---------------- qkv ----------------
    with tc.tile_pool(name="qkvps", bufs=3, space="PSUM") as qkvps:
        for tt in range(T // NQ):
            for w_sb, b_sb, dst in ((wk_sb, bkp, kt_sb), (wq_sb, bqp, qt_sb)):
                for j in range(2):
                    ps = qkvps.tile([P, NQ], F32, tag="qk")
                    for cc in range(C // P):
                        mm(
                            ps[:],
                            w_sb[:, cc, ts(j, P)],
                            xt_sb[:, cc, ts(tt, NQ)],
                            start=(cc == 0),
                            stop=(cc == C // P - 1),
                        )
                    nc.vector.tensor_scalar_add(
                        dst[:, j, ts(tt, NQ)], ps[:], b_sb[:, j : j + 1]
                    )
            for l4 in range(NQ // P):
                lc = (NQ // P) * tt + l4
                ps = qkvps.tile([P, DL], F32, tag="v")
                for cc in range(C // P):
                    mm(
                        ps[:],
                        xt_sb[:, cc, ds(P * lc, P)],
                        wv_sb[:, cc, :],
                        start=(cc == 0),
                        stop=(cc == C // P - 1),
                    )
                nc.vector.tensor_add(
                    v_sb[:, lc, :, 0:D],
                    ps[:].rearrange("p (h d) -> p h d", h=HL),
                    bv_bc[:].rearrange("p (h d) -> p h d", h=HL),
                )

    if dbg is not None:
        nc.sync.dma_start(dbg["qt"], qt_sb[:])
        nc.sync.dma_start(dbg["kt"], kt_sb[:])
        nc.sync.dma_start(dbg["v"], v_sb[:])

    # ---------------- attention ----------------
    with (
        tc.tile_pool(name="pp", bufs=3) as pp,
        tc.tile_pool(name="nrm", bufs=4) as nrm,
        tc.tile_pool(name="sps", bufs=2, space="PSUM") as sps,
        tc.tile_pool(name="ops", bufs=2, space="PSUM") as ops,
    ):
        for pair in range(2):
            for qt in range(T // NQ):
                q0 = NQ * qt
                nl = q0 // P + NQ // P  # l-chunks for causal coverage
                # [l, hi, q] -- both heads side by side (2 psum banks)
                o_ps = ops.tile([P, 2, NQ], F32, tag="o", name="o_ps")

                def s_stage(lc):
                    # both heads' scores: two row-tiled K=64 matmuls into one
                    # 2-bank psum tile; skip fully-masked columns [0, w0)
                    off = P * lc - q0
                    w0 = max(off, 0)
                    s2 = sps.tile([P, 2, NQ], F32, tag="s", name="s2")
                    for hi in range(2):
                        mm(
                            s2[:, hi, w0:NQ],
                            kt_sb[64 * hi : 64 * hi + 64, pair, ts(lc, P)],
                            qt_sb[64 * hi : 64 * hi + 64, pair, ds(q0 + w0, NQ - w0)],
                            start=True,
                            stop=True,
                            tile_position=(64 * hi, 0),
                        )
                    return s2

                def pv_stage(lc, s2):
                    off = P * lc - q0
                    w0 = max(off, 0)
                    pt = pp.tile([P, 2, NQ], BF16, tag="p", name="pt")
                    use_act = ((lc % 9) % 2 == 0) or not int(
                        os.environ.get("KERNEL_SCHRAUDOLPH", "1")
                    )
                    if use_act:
                        nc.scalar.activation(
                            pt[:, :, w0:NQ],
                            s2[:, :, w0:NQ],
                            mybir.ActivationFunctionType.Exp,
                            bias=0.0,
                            scale=SCALE,
                        )
                    else:
                        nc.vector.tensor_scalar(
                            pt[:, :, w0:NQ].bitcast(I16),
                            s2[:, :, w0:NQ],
                            A16 * SCALE,
                            B16,
                            AluOpType.mult,
                            AluOpType.add,
                        )
                    if off >= 0:
                        nc.vector.tensor_mul(
                            pt[:, :, ds(off, P)],
                            pt[:, :, ds(off, P)],
                            tri_sb[:].unsqueeze(1).broadcast_to([P, 2, P]),
                        )
                    for hi in range(2):
                        mm(
                            o_ps[:, hi, w0:NQ],
                            v_sb[:, lc, 2 * pair + hi, :],
                            pt[:, hi, w0:NQ],
                            start=(lc == 0),
                            stop=(lc == nl - 1),
                        )

                # software pipeline: keep one S stage ahead of exp/PV
                prev_lc, prev_s = 0, s_stage(0)
                for lc in range(1, nl):
                    cur = s_stage(lc)
                    pv_stage(prev_lc, prev_s)
                    prev_lc, prev_s = lc, cur
                pv_stage(prev_lc, prev_s)

                # normalization, both heads fused: sums live at psum row D
                # (reciprocal_approx_fast cannot read PSUM -- stage via SBUF)
                sums_sb = nrm.tile([1, 2, NQ], F32, tag="sums")
                nc.vector.tensor_copy(sums_sb[:], o_ps[D : D + 1, :, :])
                rcp = nrm.tile([1, 2, NQ], F32, tag="rcp")
                nc.vector.reciprocal_approx_fast(rcp[:], sums_sb[:])
                bc = nrm.tile([D, 2, NQ], F32, tag="bc")
                nc.gpsimd.partition_broadcast(bc[:], rcp[:])
                nc.vector.tensor_mul(
                    yth[:, :, pair, ds(q0, NQ)], o_ps[0:D, :, :], bc[:]
                )
                # ship this query tile's Y^T as soon as it is normalized
                for hi in range(2):
                    nc.sync.dma_start(
                        ytl[pair][ds(D * hi, D), ds(q0, NQ)],
                        yth[:, hi, pair, ds(q0, NQ)],
                    )
            nc.gpsimd.collective_compute(
                "AllGather",
                mybir.AluOpType.bypass,
                replica_groups=[[0, 1, 2, 3], [4, 5, 6, 7]],
                ins=[ytl[pair][:]],
                outs=[ytf[pair][:]],
            )

    if dbg is not None:
        nc.sync.dma_start(
            dbg["yth"], yth[:].rearrange("d hi pair t -> (hi d) pair t")
        )

    # ---------------- proj (computed transposed: out^T[o, t]) ----------------
    # out^T[o, t] = sum_c wp[c, o] y[t, c]: stationary wp chunk, moving y^T
    # tiles of 512. The 8 psum tiles accumulate pair-0 contributions right
    # after AllGather 0 (overlapping AllGather 1), then pair-1 contributions.
    with (
        tc.tile_pool(name="po", bufs=4) as po,
        tc.tile_pool(name="prps", bufs=1, space="PSUM") as prps,
    ):
        ytf_sb = []
        for pair in range(2):
            t_ = pers.tile([P, NG, T], BF16, tag=f"ytf{pair}", name=f"ytf{pair}_sb")
            for r in range(NG):
                nc.sync.dma_start(t_[:, r, :], ytf[pair][ds(P * r, P), :])
            ytf_sb.append(t_)
        pr = {}
        for o2 in range(2):
            for tq in range(T // NQ):
                pr[o2, tq] = prps.tile(
                    [P, NQ], F32, tag=f"pr{o2}{tq}", name=f"pr{o2}{tq}"
                )
        for pair in range(2):
            for o2 in range(2):
                for tq in range(T // NQ):
                    for r in range(NG):
                        mm(
                            pr[o2, tq][:],
                            wp_sb[:, 2 * r + pair, ts(o2, P)],
                            ytf_sb[pair][:, r, ts(tq, NQ)],
                            start=(pair == 0 and r == 0),
                            stop=(pair == 1 and r == NG - 1),
                        )
        for o2 in range(2):
            for tq in range(T // NQ):
                ot = po.tile([P, NQ], F32, tag="ot")
                nc.vector.tensor_scalar_add(
                    ot[:], pr[o2, tq][:], bpp[:, o2 : o2 + 1]
                )
                nc.sync.dma_start(out[ds(P * o2, P), ds(NQ * tq, NQ)], ot[:])


def build_program():
    nc = bacc.Bacc(
        "TRN2",
        target_bir_lowering=False,
        debug=False,
        enable_asserts=False,
        num_devices=NC,
    )
    xt = nc.dram_tensor("xt", [C, T], BF16, kind="ExternalInput").ap()
    wq = nc.dram_tensor("wq", [C, DL], BF16, kind="ExternalInput").ap()
    wk = nc.dram_tensor("wk", [C, DL], BF16, kind="ExternalInput").ap()
    wv = nc.dram_tensor("wv", [C, DL], BF16, kind="ExternalInput").ap()
    bq = nc.dram_tensor("bq", [DL], F32, kind="ExternalInput").ap()
    bk = nc.dram_tensor("bk", [DL], F32, kind="ExternalInput").ap()
    bv = nc.dram_tensor("bv", [DL], F32, kind="ExternalInput").ap()
    wp = nc.dram_tensor("wp", [C, DL], BF16, kind="ExternalInput").ap()
    bp = nc.dram_tensor("bp", [DL], F32, kind="ExternalInput").ap()
    tri = nc.dram_tensor("tri", [P, P], BF16, kind="ExternalInput").ap()
    out = nc.dram_tensor("out", [DL, T], F32, kind="ExternalOutput").ap()
    ytl = [
        nc.dram_tensor(f"ytl{p}", [DL // 2, T], BF16, kind="Internal").ap()
        for p in range(2)
    ]
    ytf = [
        nc.dram_tensor(f"ytf{p}", [NG * DL // 2, T], BF16, kind="Internal").ap()
        for p in range(2)
    ]
    io = (xt, wq, wk, wv, bq, bk, bv, wp, bp, tri, out, ytl, ytf)
    dbg = None
    if int(os.environ.get("KERNEL_DEBUG", "0")):
        dbg = {
            "qt": nc.dram_tensor("dbg_qt", [P, 2, T], BF16, kind="ExternalOutput").ap(),
            "kt": nc.dram_tensor("dbg_kt", [P, 2, T], BF16, kind="ExternalOutput").ap(),
            "v": nc.dram_tensor(
                "dbg_v", [P, T // P, HL, P], BF16, kind="ExternalOutput"
            ).ap(),
            "yth": nc.dram_tensor(
                "dbg_yth", [P, 2, T], BF16, kind="ExternalOutput"
            ).ap(),
            "sums": nc.dram_tensor(
                "dbg_sums", [16, NQ], F32, kind="ExternalOutput"
            ).ap(),
            "rcp": nc.dram_tensor(
                "dbg_rcp", [16, NQ], F32, kind="ExternalOutput"
            ).ap(),
            "o": nc.dram_tensor("dbg_o", [D, NQ], F32, kind="ExternalOutput").ap(),
        }
    with tile.TileContext(nc) as tc:
        import contextlib

        with contextlib.ExitStack() as ctx:
            _build_body(ctx, tc, io, dbg)
    nc.compile()
    return nc


def make_in_maps(x, W_attn, b_attn, W_proj, b_proj):
    # scores are computed transposed (S^T[l, q]); position (l', q'') in a
    # diagonal 128x128 block is causally valid iff q'' >= l' -> upper-tri mask
    tri_np = np.triu(np.ones((P, P), dtype=np.float32)).astype(ml_dtypes.bfloat16)
    x = np.asarray(x, dtype=np.float32)
    W_attn = np.asarray(W_attn, dtype=np.float32)
    b_attn = np.asarray(b_attn, dtype=np.float32)
    W_proj = np.asarray(W_proj, dtype=np.float32)
    b_proj = np.asarray(b_proj, dtype=np.float32)
    bf = lambda a: np.ascontiguousarray(a).astype(ml_dtypes.bfloat16)
    in_maps = []
    for c in range(NC):
        b, g = divmod(c, NG)
        cols = slice(DL * g, DL * (g + 1))
        in_maps.append(
            {
                "xt": bf(x[b].T),
                "wq": bf(W_attn[:, cols]),
                "wk": bf(W_attn[:, C:][:, cols]),
                "wv": bf(W_attn[:, 2 * C :][:, cols]),
                "bq": np.ascontiguousarray(b_attn[cols]),
                "bk": np.ascontiguousarray(b_attn[C:][cols]),
                "bv": np.ascontiguousarray(b_attn[2 * C :][cols]),
                "wp": bf(W_proj[:, cols]),
                "bp": np.ascontiguousarray(b_proj[cols]),
                "tri": tri_np,
            }
        )
    return in_maps


_NC_CACHE = {}


def _install_ntff_hook():
    """Recreate the missing antenv.axon_hooks module so
    run_bass_kernel_spmd(trace=True) can capture NTFF profiles under axon."""
    import sys
    import types

    if "antenv.axon_hooks" in sys.modules:
        return True
    try:
        from trn_agent_boot.trn_boot import _ntff_profile_via_ctypes

        hook = _ntff_profile_via_ctypes("/opt/axon/libaxon_pjrt.so")
        if hook is None:
            return False
        mod = types.ModuleType("antenv.axon_hooks")
        mod.get_axon_ntff_profile_hook = lambda: hook
        mod.set_axon_ntff_profile_hook = lambda h: None
        sys.modules["antenv.axon_hooks"] = mod
        import antenv

        antenv.axon_hooks = mod
        # the trace path uploads artifacts to a fish bucket that doesn't
        # exist in this container; keep them local instead
        bass_utils.upload_artifacts = lambda tmpdir: tmpdir
        return True
    except Exception:
        return False


def _get_program():
    if "nc" not in _NC_CACHE:
        nc = build_program()
        nc.m = get_hw_module(nc.m)
        _NC_CACHE["nc"] = nc
    return _NC_CACHE["nc"]


def kernel(x, W_attn, b_attn, W_proj, b_proj):
    nc = _get_program()
    in_maps = make_in_maps(x, W_attn, b_attn, W_proj, b_proj)
    trace = bool(int(os.environ.get("KERNEL_TRACE", "0")))
    if trace:
        trace = _install_ntff_hook()
    res = bass_utils.run_bass_kernel_spmd(
        nc,
        in_maps,
        core_ids=list(range(NC)),
        trace=trace,
        trace_cores=list(range(NC)) if trace else None,
    )
    if trace:
        _NC_CACHE["last_results"] = res
        if res.exec_time_ns is not None:
            print(f"HW exec time: {res.exec_time_ns} ns")
            if res.instructions_and_trace is not None:
                print(f"trace: {res.instructions_and_trace[1]}")
    out = np.empty((B, T, C), dtype=np.float32)
    for c in range(NC):
        b, g = divmod(c, NG)
        out[b, :, DL * g : DL * (g + 1)] = res.results[c]["out"].T
    return out


# revision 20
# speedup vs baseline: 1.3913x; 1.3913x over previous
"""Causal self-attention (B=2, T=2048, C=1024, H=16, D=64) on 8 trn2 NeuronCores.

Sharding: batch x head-group. Core c handles batch b = c//4 and head group
g = c%4 (4 heads = 256 channels). All matmul operands are bf16 (fp32 PSUM
accumulation); host pre-casts inputs to bf16, halving DMA and enabling FWL
so LDWEIGHTS pipelines behind matmuls.

Per core:
  - qkv projection for its 4 heads (Q^T/K^T in [d, t] layout, V in
    [t, d|ones|pad] layout padded to 128 weight columns for FWL)
  - causal flash attention, K-major scores S^T. Both heads of a pair are
    computed per 128-key chunk: two row-tiled (K=64) score matmuls into one
    2-bank PSUM tile, then ONE exp over [128, 2*512]:
      even chunks: ACT engine true exp (bf16 out)
      odd  chunks: DVE Schraudolph bit-trick exp (fp32->int16 mult+add,
                   int16 bits == bf16 exp approx, ~3% elementwise, washes
                   out through softmax normalization; end-to-end emulated
                   rel err 3.4e-3 vs 2e-2 budget)
    row sums via a ones-column appended to V; no max subtraction (logits
    are O(1) for this problem family).
  - AllGather of Y^T (bf16) across the 4 cores of the batch, one per head
    pair; first overlaps attention of the second pair.
  - output projection for a 256-column slice, accumulated pair-by-pair.

Host gather: concatenate the 4 column slices per batch. No host math.
"""

import os
import numpy as np
import ml_dtypes

import concourse.bass as bass
import concourse.bacc as bacc
import concourse.mybir as mybir
import concourse.tile as tile
from concourse import bass_utils
from concourse.bass import ds, ts
from concourse.alu_op_type import AluOpType
from concourse.bass_interp import get_hw_module

P = 128
B, T, C = 2, 2048, 1024
NH, D = 16, 64
NC = 8          # cores
NG = 4          # head groups (cores per batch)
HL = NH // NG   # heads per core = 4
DL = HL * D     # local channels = 256
NQ = 512        # query tile
F32 = mybir.dt.float32
BF16 = mybir.dt.bfloat16
I16 = mybir.dt.int16

SCALE = 1.0 / np.sqrt(D)
# Schraudolph fast-exp in bf16 bit space: bf16(exp(x)) bits ~= round(A*x + Bc)
A16 = 128.0 / np.log(2.0)
B16 = 127.0 * 128.0 - 5.5


def _build_body(ctx, tc, io, dbg=None):
    nc = tc.nc
    xt, wq, wk, wv, bq, bk, bv, wp, bp, tri, out, ytl, ytf = io
    mm = nc.tensor.matmul

    pers = ctx.enter_context(tc.tile_pool(name="pers", bufs=1))

    # ---------------- persistent SBUF + input loads ----------------
    tri_sb = pers.tile([P, P], BF16)
    nc.sync.dma_start(tri_sb[:], tri)

    qt_sb = pers.tile([P, 2, T], BF16)   # chunk j = head pair j
    kt_sb = pers.tile([P, 2, T], BF16)
    # V: [l_part, l_chunk, head, d | ones | zero-pad] -- 128 weight cols
    v_sb = pers.tile([P, T // P, HL, P], BF16)
    nc.vector.memset(v_sb[:, :, :, D:], 0.0)
    nc.vector.memset(v_sb[:, :, :, D : D + 1], 1.0)
    # [d, hi, pair, t]
    yth = pers.tile([D, 2, 2, T], BF16)

    xt_sb = pers.tile([P, C // P, T], BF16)
    wq_sb = pers.tile([P, C // P, DL], BF16)
    wk_sb = pers.tile([P, C // P, DL], BF16)
    wv_sb = pers.tile([P, C // P, DL], BF16)
    nc.sync.dma_start(wk_sb[:], wk.rearrange("(o p) n -> p o n", p=P))
    nc.sync.dma_start(wq_sb[:], wq.rearrange("(o p) n -> p o n", p=P))
    nc.sync.dma_start(wv_sb[:], wv.rearrange("(o p) n -> p o n", p=P))
    # T-tile-major so the first matmul groups complete after ~1/4 of the load
    for tt in range(T // NQ):
        nc.sync.dma_start(
            xt_sb[:, :, ts(tt, NQ)],
            xt[:, ds(NQ * tt, NQ)].rearrange("(cc p) t -> p cc t", p=P),
        )

    bqp = pers.tile([P, 2], F32)
    nc.sync.dma_start(bqp[:], bq.rearrange("(j p) -> p j", p=P))
    bkp = pers.tile([P, 2], F32)
    nc.sync.dma_start(bkp[:], bk.rearrange("(j p) -> p j", p=P))
    bv_row = pers.tile([1, DL], F32)
    nc.sync.dma_start(bv_row[:], bv[None, :])
    bv_bc = pers.tile([P, DL], F32)
    nc.gpsimd.partition_broadcast(bv_bc[:], bv_row[:])

    wp_sb = pers.tile([P, C // P, DL], BF16)
    nc.sync.dma_start(wp_sb[:], wp.rearrange("(o p) n -> p o n", p=P))
    bpp = pers.tile([P, 2], F32)
    nc.sync.dma_start(bpp[:], bp.rearrange("(j p) -> p j", p=P))

    # preload the exp table set and warm the PE clock gate while the input
    # DMAs stream: junk activations + matmuls on already-resident tiles
    with (
        tc.tile_pool(name="warm", bufs=2) as warm,
        tc.tile_pool(name="warmps", bufs=2, space="PSUM") as warmps,
    ):
        wact = warm.tile([1, 32], F32, name="wact")
        nc.scalar.activation(
            wact[:], tri_sb[0:1, 0:32], mybir.ActivationFunctionType.Exp,
            bias=0.0, scale=1.0,
        )
        vwarm = v_sb[:, 0, :, :].rearrange("p h d -> p (h d)")
        for i in range(24):
            wps = warmps.tile([P, NQ], F32, tag="w", name="wps")
            mm(wps[:], tri_sb[:], vwarm[:, 0:NQ], start=True, stop=True)

    # ---------------- qkv ----------------
    with tc.tile_pool(name="qkvps", bufs=3, space="PSUM") as qkvps:
        for tt in range(T // NQ):
            for w_sb, b_sb, dst in ((wk_sb, bkp, kt_sb), (wq_sb, bqp, qt_sb)):
                for j in range(2):
                    ps = qkvps.tile([P, NQ], F32, tag="qk")
                    for cc in range(C // P):
                        mm(
                            ps[:],
                            w_sb[:, cc, ts(j, P)],
                            xt_sb[:, cc, ts(tt, NQ)],
                            start=(cc == 0),
                            stop=(cc == C // P - 1),
                        )
                    nc.vector.tensor_scalar_add(
                        dst[:, j, ts(tt, NQ)], ps[:], b_sb[:, j : j + 1]
                    )
            for l4 in range(NQ // P):
                lc = (NQ // P) * tt + l4
                ps = qkvps.tile([P, DL], F32, tag="v")
                for cc in range(C // P):
                    mm(
                        ps[:],
                        xt_sb[:, cc, ds(P * lc, P)],
                        wv_sb[:, cc, :],
                        start=(cc == 0),
                        stop=(cc == C // P - 1),
                    )
                nc.vector.tensor_add(
                    v_sb[:, lc, :, 0:D],
                    ps[:].rearrange("p (h d) -> p h d", h=HL),
                    bv_bc[:].rearrange("p (h d) -> p h d", h=HL),
                )

    if dbg is not None:
        nc.sync.dma_start(dbg["qt"], qt_sb[:])
        nc.sync.dma_start(dbg["kt"], kt_sb[:])
        nc.sync.dma_start(dbg["v"], v_sb[:])

    # ---------------- attention ----------------
    with (
        tc.tile_pool(name="pp", bufs=3) as pp,
        tc.tile_pool(name="nrm", bufs=4) as nrm,
        tc.tile_pool(name="sps", bufs=2, space="PSUM") as sps,
        tc.tile_pool(name="ops", bufs=2, space="PSUM") as ops,
    ):
        for pair in range(2):
            for qt in range(T // NQ):
                q0 = NQ * qt
                nl = q0 // P + NQ // P  # l-chunks for causal coverage
                # [l, hi, q] -- both heads side by side (2 psum banks)
                o_ps = ops.tile([P, 2, NQ], F32, tag="o", name="o_ps")

                def s_stage(lc):
                    # both heads' scores: two row-tiled K=64 matmuls into one
                    # 2-bank psum tile; skip fully-masked columns [0, w0)
                    off = P * lc - q0
                    w0 = max(off, 0)
                    s2 = sps.tile([P, 2, NQ], F32, tag="s", name="s2")
                    for hi in range(2):
                        mm(
                            s2[:, hi, w0:NQ],
                            kt_sb[64 * hi : 64 * hi + 64, pair, ts(lc, P)],
                            qt_sb[64 * hi : 64 * hi + 64, pair, ds(q0 + w0, NQ - w0)],
                            start=True,
                            stop=True,
                            tile_position=(64 * hi, 0),
                        )
                    return s2

                def pv_stage(lc, s2):
                    off = P * lc - q0
                    w0 = max(off, 0)
                    pt = pp.tile([P, 2, NQ], BF16, tag="p", name="pt")
                    use_act = ((lc % 9) % 2 == 0) or not int(
                        os.environ.get("KERNEL_SCHRAUDOLPH", "1")
                    )
                    if use_act:
                        nc.scalar.activation(
                            pt[:, :, w0:NQ],
                            s2[:, :, w0:NQ],
                            mybir.ActivationFunctionType.Exp,
                            bias=0.0,
                            scale=SCALE,
                        )
                    else:
                        nc.vector.tensor_scalar(
                            pt[:, :, w0:NQ].bitcast(I16),
                            s2[:, :, w0:NQ],
                            A16 * SCALE,
                            B16,
                            AluOpType.mult,
                            AluOpType.add,
                        )
                    if off >= 0:
                        nc.vector.tensor_mul(
                            pt[:, :, ds(off, P)],
                            pt[:, :, ds(off, P)],
                            tri_sb[:].unsqueeze(1).broadcast_to([P, 2, P]),
                        )
                    for hi in range(2):
                        mm(
                            o_ps[:, hi, w0:NQ],
                            v_sb[:, lc, 2 * pair + hi, :],
                            pt[:, hi, w0:NQ],
                            start=(lc == 0),
                            stop=(lc == nl - 1),
                        )

                # software pipeline: keep one S stage ahead of exp/PV
                prev_lc, prev_s = 0, s_stage(0)
                for lc in range(1, nl):
                    cur = s_stage(lc)
                    pv_stage(prev_lc, prev_s)
                    prev_lc, prev_s = lc, cur
                pv_stage(prev_lc, prev_s)

                # normalization, both heads fused: sums live at psum row D
                # (reciprocal_approx_fast cannot read PSUM -- stage via SBUF)
                sums_sb = nrm.tile([1, 2, NQ], F32, tag="sums")
                nc.vector.tensor_copy(sums_sb[:], o_ps[D : D + 1, :, :])
                rcp = nrm.tile([1, 2, NQ], F32, tag="rcp")
                nc.vector.reciprocal_approx_fast(rcp[:], sums_sb[:])
                bc = nrm.tile([D, 2, NQ], F32, tag="bc")
                nc.gpsimd.partition_broadcast(bc[:], rcp[:])
                nc.vector.tensor_mul(
                    yth[:, :, pair, ds(q0, NQ)], o_ps[0:D, :, :], bc[:]
                )
                # ship this query tile's Y^T as soon as it is normalized
                for hi in range(2):
                    nc.sync.dma_start(
                        ytl[pair][ds(D * hi, D), ds(q0, NQ)],
                        yth[:, hi, pair, ds(q0, NQ)],
                    )
            nc.gpsimd.collective_compute(
                "AllGather",
                mybir.AluOpType.bypass,
                replica_groups=[[0, 1, 2, 3], [4, 5, 6, 7]],
                ins=[ytl[pair][:]],
                outs=[ytf[pair][:]],
            )

    if dbg is not None:
        nc.sync.dma_start(
            dbg["yth"], yth[:].rearrange("d hi pair t -> (hi d) pair t")
        )

    # ---------------- proj (computed transposed: out^T[o, t]) ----------------
    # out^T[o, t] = sum_c wp[c, o] y[t, c]: stationary wp chunk, moving y^T
    # tiles of 512. The 8 psum tiles accumulate pair-0 contributions right
    # after AllGather 0 (overlapping AllGather 1), then pair-1 contributions.
    with (
        tc.tile_pool(name="po", bufs=4) as po,
        tc.tile_pool(name="prps", bufs=1, space="PSUM") as prps,
    ):
        ytf_sb = []
        for pair in range(2):
            t_ = pers.tile([P, NG, T], BF16, tag=f"ytf{pair}", name=f"ytf{pair}_sb")
            for r in range(NG):
                nc.sync.dma_start(t_[:, r, :], ytf[pair][ds(P * r, P), :])
            ytf_sb.append(t_)
        pr = {}
        for o2 in range(2):
            for tq in range(T // NQ):
                pr[o2, tq] = prps.tile(
                    [P, NQ], F32, tag=f"pr{o2}{tq}", name=f"pr{o2}{tq}"
                )
        for pair in range(2):
            for o2 in range(2):
                for tq in range(T // NQ):
                    for r in range(NG):
                        mm(
                            pr[o2, tq][:],
                            wp_sb[:, 2 * r + pair, ts(o2, P)],
                            ytf_sb[pair][:, r, ts(tq, NQ)],
                            start=(pair == 0 and r == 0),
                            stop=(pair == 1 and r == NG - 1),
                        )
        for o2 in range(2):
            for tq in range(T // NQ):
                ot = po.tile([P, NQ], F32, tag="ot")
                nc.vector.tensor_scalar_add(
                    ot[:], pr[o2, tq][:], bpp[:, o2 : o2 + 1]
                )
                nc.sync.dma_start(out[ds(P * o2, P), ds(NQ * tq, NQ)], ot[:])


def build_program():
    nc = bacc.Bacc(
        "TRN2",
        target_bir_lowering=False,
        debug=False,
        enable_asserts=False,
        num_devices=NC,
    )
    xt = nc.dram_tensor("xt", [C, T], BF16, kind="ExternalInput").ap()
    wq = nc.dram_tensor("wq", [C, DL], BF16, kind="ExternalInput").ap()
    wk = nc.dram_tensor("wk", [C, DL], BF16, kind="ExternalInput").ap()
    wv = nc.dram_tensor("wv", [C, DL], BF16, kind="ExternalInput").ap()
    bq = nc.dram_tensor("bq", [DL], F32, kind="ExternalInput").ap()
    bk = nc.dram_tensor("bk", [DL], F32, kind="ExternalInput").ap()
    bv = nc.dram_tensor("bv", [DL], F32, kind="ExternalInput").ap()
    wp = nc.dram_tensor("wp", [C, DL], BF16, kind="ExternalInput").ap()
    bp = nc.dram_tensor("bp", [DL], F32, kind="ExternalInput").ap()
    tri = nc.dram_tensor("tri", [P, P], BF16, kind="ExternalInput").ap()
    out = nc.dram_tensor("out", [DL, T], F32, kind="ExternalOutput").ap()
    ytl = [
        nc.dram_tensor(f"ytl{p}", [DL // 2, T], BF16, kind="Internal").ap()
        for p in range(2)
    ]
    ytf = [
        nc.dram_tensor(f"ytf{p}", [NG * DL // 2, T], BF16, kind="Internal").ap()
        for p in range(2)
    ]
    io = (xt, wq, wk, wv, bq, bk, bv, wp, bp, tri, out, ytl, ytf)
    dbg = None
    if int(os.environ.get("KERNEL_DEBUG", "0")):
        dbg = {
            "qt": nc.dram_tensor("dbg_qt", [P, 2, T], BF16, kind="ExternalOutput").ap(),
            "kt": nc.dram_tensor("dbg_kt", [P, 2, T], BF16, kind="ExternalOutput").ap(),
            "v": nc.dram_tensor(
                "dbg_v", [P, T // P, HL, P], BF16, kind="ExternalOutput"
            ).ap(),
            "yth": nc.dram_tensor(
                "dbg_yth", [P, 2, T], BF16, kind="ExternalOutput"
            ).ap(),
            "sums": nc.dram_tensor(
                "dbg_sums", [16, NQ], F32, kind="ExternalOutput"
            ).ap(),
            "rcp": nc.dram_tensor(
                "dbg_rcp", [16, NQ], F32, kind="ExternalOutput"
            ).ap(),
            "o": nc.dram_tensor("dbg_o", [D, NQ], F32, kind="ExternalOutput").ap(),
        }
    with tile.TileContext(nc) as tc:
        import contextlib

        with contextlib.ExitStack() as ctx:
            _build_body(ctx, tc, io, dbg)
    nc.compile()
    return nc


def make_in_maps(x, W_attn, b_attn, W_proj, b_proj):
    # scores are computed transposed (S^T[l, q]); position (l', q'') in a
    # diagonal 128x128 block is causally valid iff q'' >= l' -> upper-tri mask
    tri_np = np.triu(np.ones((P, P), dtype=np.float32)).astype(ml_dtypes.bfloat16)
    x = np.asarray(x, dtype=np.float32)
    W_attn = np.asarray(W_attn, dtype=np.float32)
    b_attn = np.asarray(b_attn, dtype=np.float32)
    W_proj = np.asarray(W_proj, dtype=np.float32)
    b_proj = np.asarray(b_proj, dtype=np.float32)
    bf = lambda a: np.ascontiguousarray(a).astype(ml_dtypes.bfloat16)
    in_maps = []
    for c in range(NC):
        b, g = divmod(c, NG)
        cols = slice(DL * g, DL * (g + 1))
        in_maps.append(
            {
                "xt": bf(x[b].T),
                "wq": bf(W_attn[:, cols]),
                "wk": bf(W_attn[:, C:][:, cols]),
                "wv": bf(W_attn[:, 2 * C :][:, cols]),
                "bq": np.ascontiguousarray(b_attn[cols]),
                "bk": np.ascontiguousarray(b_attn[C:][cols]),
                "bv": np.ascontiguousarray(b_attn[2 * C :][cols]),
                "wp": bf(W_proj[:, cols]),
                "bp": np.ascontiguousarray(b_proj[cols]),
                "tri": tri_np,
            }
        )
    return in_maps


_NC_CACHE = {}


def _install_ntff_hook():
    """Recreate the missing antenv.axon_hooks module so
    run_bass_kernel_spmd(trace=True) can capture NTFF profiles under axon."""
    import sys
    import types

    if "antenv.axon_hooks" in sys.modules:
        return True
    try:
        from trn_agent_boot.trn_boot import _ntff_profile_via_ctypes

        hook = _ntff_profile_via_ctypes("/opt/axon/libaxon_pjrt.so")
        if hook is None:
            return False
        mod = types.ModuleType("antenv.axon_hooks")
        mod.get_axon_ntff_profile_hook = lambda: hook
        mod.set_axon_ntff_profile_hook = lambda h: None
        sys.modules["antenv.axon_hooks"] = mod
        import antenv

        antenv.axon_hooks = mod
        # the trace path uploads artifacts to a fish bucket that doesn't
        # exist in this container; keep them local instead
        bass_utils.upload_artifacts = lambda tmpdir: tmpdir
        return True
    except Exception:
        return False


def _get_program():
    if "nc" not in _NC_CACHE:
        nc = build_program()
        nc.m = get_hw_module(nc.m)
        _NC_CACHE["nc"] = nc
    return _NC_CACHE["nc"]


def kernel(x, W_attn, b_attn, W_proj, b_proj):
    nc = _get_program()
    in_maps = make_in_maps(x, W_attn, b_attn, W_proj, b_proj)
    trace = bool(int(os.environ.get("KERNEL_TRACE", "0")))
    if trace:
        trace = _install_ntff_hook()
    res = bass_utils.run_bass_kernel_spmd(
        nc,
        in_maps,
        core_ids=list(range(NC)),
        trace=trace,
        trace_cores=list(range(NC)) if trace else None,
    )
    if trace:
        _NC_CACHE["last_results"] = res
        if res.exec_time_ns is not None:
            print(f"HW exec time: {res.exec_time_ns} ns")
            if res.instructions_and_trace is not None:
                print(f"trace: {res.instructions_and_trace[1]}")
    out = np.empty((B, T, C), dtype=np.float32)
    for c in range(NC):
        b, g = divmod(c, NG)
        out[b, :, DL * g : DL * (g + 1)] = res.results[c]["out"].T
    return out
